# revision 27
# baseline (speedup 1.0000x reference)
"""KNN palette-retrieval kernel for Trainium2 (8 NeuronCores, data-parallel).

Per pixel of rgb_mask [16,3,512,512]: find the palette row (of 21,
L2-normalized) with max cosine similarity, emit that normalized color;
zero pixels emit 0.  argmax(cos) == argmax(dot) since pixel norm is a
positive scalar, so pixel normalization is skipped.

Layout: each core takes 2 batches = 524288 px, split into 32 "sets" g of
16384 px.  PE row layout = 32*k' + g (quadrant-aligned so every DVE
partition range starts at 0/32/64/96).  24 k-slots = 6 matmuls x 4 slots
(21 real + 3 zero-padded; sims are strictly positive so pads never win).

Sims are computed in bf16 hi/lo 3-pass form (x ~ xh+xl, cn ~ ch+cl, all
bf16, split host-side): s = ch.xh + cl.xh + ch.xl accumulated in fp32
PSUM -- ~2^-18 relative accuracy, 3*512 PE cycles per bank instead of
fp32's 4*512, and bf16 weight loads get FWL.

Per tile of 32x512 pixels (six pa banks rotate over seven PSUM pools;
the spare bank lets tile t+1's first mm1 group start while tile t's
max/one-hot tail is still consuming the other banks):
  mm1_i (x6 banks, x3 passes): pa_i = sims         (PE bf16, K=96)
  chain:  s = copy(pa_0) (ACT);  4x running TT-max vs pa_1..pa_4 (DVE)
  fold:   pa_5 as the PSUM operand folds 64 rows across bases, then two
          32-row merges; the last is a scalar_tensor_tensor fusing the
          1e-20 floor and writing into mrep[0:32]; two copies replicate
          it to mrep[32:64] and mrep[64:128].
  bank 0:  mm2 (PE fp32 accumulate, exact: winner row becomes +0.0),
          then oh_0 = Relu(pa_0 * 2.2e7 + 1) on ACT -> bf16 {0,1}.
  banks 1-5: oh_i = (pa_i is_ge mrep) on DVE -> bf16 {0,1} (exact
          compare against the fp32 max; no subtract needed).
  mm3_i (x6): pout[32c+g] += sum cn[k,c]*oh_i  (bf16 PE) -> palette color
  yout:   copy pout -> SBUF (ACT), DMA out.
"""

import sys

sys.path.insert(0, "/opt/trn_rl_repo")

import numpy as np

B, C, H, W = 16, 3, 512, 512
K = 21
NCORES = 8
BPC = B // NCORES            # batches per core
PXC = BPC * H * W            # pixels per core = 524288
G = 32                       # pixel sets (partition-packed)
REG = PXC // G               # 16384 columns per set
NT = 512                     # pixel columns per tile
NTILES = REG // NT           # 32
NMM = 6                      # k-slot matmuls (6*4 = 24 >= 21)
NPOOL = 7                    # PSUM pools for the 6 pa banks: the spare
                             # bank lets tile t+1's mm1 start during
                             # tile t's max/one-hot tail
NSUB = 1                     # banks 0..NSUB-1 use PE subtract + ACT Relu
                             # (the rest compare on DVE; 1 measured best)
OH_SCALE = 2.2e7             # 1 ulp at |m|>=0.25 clears the Relu knee

_CACHE: dict = {}


def _build_nc():
    if "nc" in _CACHE:
        return _CACHE["nc"]
    from contextlib import ExitStack

    import concourse.tile as tile
    from concourse import bacc, mybir

    f32 = mybir.dt.float32
    bf16 = mybir.dt.bfloat16
    mx = mybir.AluOpType.max
    ge = mybir.AluOpType.is_ge
    relu = mybir.ActivationFunctionType.Relu

    nc = bacc.Bacc("TRN2", target_bir_lowering=False, debug=False,
                   num_devices=NCORES)
    xh = nc.dram_tensor("xh", [C * G, REG], bf16, kind="ExternalInput").ap()
    xl = nc.dram_tensor("xl", [C * G, REG], bf16, kind="ExternalInput").ap()
    w1h = nc.dram_tensor("w1h", [NMM, C * G, 128], bf16,
                         kind="ExternalInput").ap()
    w1l = nc.dram_tensor("w1l", [NMM, C * G, 128], bf16,
                         kind="ExternalInput").ap()
    if NSUB:
        w2 = nc.dram_tensor("w2", [G, 128], f32, kind="ExternalInput").ap()
    w3 = nc.dram_tensor("w3", [NMM, 128, C * G], bf16,
                        kind="ExternalInput").ap()
    y = nc.dram_tensor("y", [C * G, REG], f32, kind="ExternalOutput").ap()

    with ExitStack() as ctx:
        tc = ctx.enter_context(tile.TileContext(nc))
        wp = ctx.enter_context(tc.tile_pool(name="w", bufs=1))
        inp = ctx.enter_context(tc.tile_pool(name="xin", bufs=3))
        sp = ctx.enter_context(tc.tile_pool(name="s", bufs=3))
        rp = ctx.enter_context(tc.tile_pool(name="r", bufs=3))
        ohp = ctx.enter_context(tc.tile_pool(name="oh", bufs=3))
        yp = ctx.enter_context(tc.tile_pool(name="y", bufs=3))
        pap = [ctx.enter_context(
            tc.tile_pool(name=f"pa{p}", bufs=1, space="PSUM"))
            for p in range(NPOOL)]
        pop = ctx.enter_context(tc.tile_pool(name="po", bufs=1, space="PSUM"))

        # mm1 weights first (gate tile 0's matmuls); w2/w3 are only needed
        # several microseconds in, so they load behind the first xh/xl.
        w1hs, w1ls, w3s = [], [], []
        for i in range(NMM):
            w1ht = wp.tile([C * G, 128], bf16, name=f"w1hs{i}")
            nc.sync.dma_start(w1ht[:], w1h[i])
            w1hs.append(w1ht)
            w1lt = wp.tile([C * G, 128], bf16, name=f"w1ls{i}")
            nc.sync.dma_start(w1lt[:], w1l[i])
            w1ls.append(w1lt)
        if NSUB:
            w2s = wp.tile([G, 128], f32)
            nc.sync.dma_start(w2s[:], w2[:])
        for i in range(NMM):
            w3t = wp.tile([128, C * G], bf16, name=f"w3s{i}")
            nc.sync.dma_start(w3t[:], w3[i])
            w3s.append(w3t)

        for t in range(NTILES):
            c0 = t * NT
            xht = inp.tile([C * G, NT], bf16, tag="xh")
            nc.sync.dma_start(xht[:], xh[:, c0:c0 + NT])
            xlt = inp.tile([C * G, NT], bf16, tag="xl")
            nc.sync.dma_start(xlt[:], xl[:, c0:c0 + NT])

            pa = []
            for i in range(NMM):
                p = (NMM * t + i) % NPOOL
                pai = pap[p].tile([128, NT], f32, tag=f"pa{p}",
                                  name=f"pa{p}_t{t}")
                nc.tensor.matmul(pai[:], w1hs[i][:], xht[:],
                                 start=True, stop=False)
                nc.tensor.matmul(pai[:], w1ls[i][:], xht[:],
                                 start=False, stop=False)
                nc.tensor.matmul(pai[:], w1hs[i][:], xlt[:],
                                 start=False, stop=False)
                pa.append(pai)

            # running max chain over pa_0..4 (<=1 PSUM operand per TT;
            # SBUF+SBUF inputs must share base partition, PSUM+SBUF is free)
            s = sp.tile([128, NT], f32, tag="s")
            sm = sp.tile([128, NT], f32, tag="sm")
            nc.scalar.copy(s[:], pa[0][:])
            nc.vector.tensor_tensor(sm[:], pa[1][:], s[:], mx)
            nc.vector.tensor_tensor(s[:], pa[2][:], sm[:], mx)
            nc.vector.tensor_tensor(sm[:], pa[3][:], s[:], mx)
            nc.vector.tensor_tensor(s[:], pa[4][:], sm[:], mx)
            # fold: pa_5 (1 real + 3 zero slots) as the PSUM operand lets
            # the 64-row fold cross bases; m12 merges the two 64-row
            # operands at equal base 0 in one TT; a cheap base-rebasing
            # copy then lines up its halves for the floor-fused STT.
            u = sp.tile([64, NT], f32, tag="u")
            nc.vector.tensor_tensor(u[:], pa[5][0:64, :], s[64:128, :], mx)
            m12 = sp.tile([64, NT], f32, tag="m12")
            nc.vector.tensor_tensor(m12[:], u[:], s[0:64, :], mx)
            mc = sp.tile([32, NT], f32, tag="mc")
            nc.scalar.copy(mc[:], m12[32:64, :])
            mrep = rp.tile([128, NT], f32, tag="mrep")
            nc.vector.scalar_tensor_tensor(mrep[0:32, :], mc[:], 1e-20,
                                           m12[0:32, :], mx, mx)
            nc.vector.tensor_copy(mrep[32:64, :], mrep[0:32, :])
            nc.scalar.copy(mrep[64:128, :], mrep[0:64, :])

            for i in range(NSUB):
                nc.tensor.matmul(pa[i][:], w2s[:], mrep[0:32, :],
                                 start=False, stop=True)

            # mm3 bank order: is_ge banks first (their oh is ready as soon
            # as mrep lands, before the mm2+Relu banks).
            pout = pop.tile([C * G, NT], f32, tag="po")
            order = list(range(NSUB, NMM)) + list(range(NSUB))
            for j, i in enumerate(order):
                oh = ohp.tile([128, NT], bf16, tag=f"oh{i}", name=f"oh{i}")
                if i < NSUB:
                    nc.scalar.activation(oh[:], pa[i][:], relu,
                                         bias=1.0, scale=OH_SCALE)
                else:
                    nc.vector.tensor_tensor(oh[:], pa[i][:], mrep[:], ge)
                nc.tensor.matmul(pout[:], w3s[i][:], oh[:],
                                 start=(j == 0), stop=(j == NMM - 1))

            yout = yp.tile([C * G, NT], f32, tag="yout")
            nc.scalar.copy(yout[:], pout[:])
            nc.sync.dma_start(y[:, c0:c0 + NT], yout[:])

    nc.compile()
    _CACHE["nc"] = nc
    return nc


def _weights(colors: np.ndarray):
    import ml_dtypes

    bf = ml_dtypes.bfloat16
    cn = (colors.astype(np.float64)
          / np.linalg.norm(colors.astype(np.float64), axis=-1, keepdims=True))
    W1 = np.zeros((NMM, C * G, 128), np.float32)
    W2 = np.zeros((G, 128), np.float32)
    W3 = np.zeros((NMM, 128, C * G), np.float32)
    for i in range(NMM):
        for kp in range(4):
            k = 4 * i + kp
            if k >= K:
                continue
            for g in range(G):
                for c in range(C):
                    W1[i, G * c + g, G * kp + g] = cn[k, c]
                    W3[i, G * kp + g, G * c + g] = cn[k, c]
    for g in range(G):
        for kp in range(4):
            W2[g, G * kp + g] = -1.0
    W1h = W1.astype(bf)
    W1l = (W1 - W1h.astype(np.float32)).astype(bf)
    return W1h, W1l, W2, W3.astype(bf)


def _stage_inputs(rgb_mask: np.ndarray, colors: np.ndarray):
    import ml_dtypes

    bf = ml_dtypes.bfloat16
    W1h, W1l, W2, W3 = _weights(np.asarray(colors, np.float32))
    in_maps = []
    for i in range(NCORES):
        xc = np.asarray(rgb_mask[BPC * i:BPC * (i + 1)], np.float32)
        xc = np.ascontiguousarray(
            np.transpose(xc, (1, 0, 2, 3)).reshape(C * G, REG))
        xh = xc.astype(bf)
        xl = (xc - xh.astype(np.float32)).astype(bf)
        im = {"xh": xh, "xl": xl, "w1h": W1h, "w1l": W1l, "w3": W3}
        if NSUB:
            im["w2"] = W2
        in_maps.append(im)
    return in_maps


def _gather_outputs(results):
    outs = []
    for i in range(NCORES):
        yb = results[i]["y"].reshape(C, BPC, H, W)
        outs.append(np.transpose(yb, (1, 0, 2, 3)))
    return np.ascontiguousarray(np.concatenate(outs, axis=0))


def run(rgb_mask, colors, trace=False, **kw):
    from concourse.bass_utils import run_bass_kernel_spmd

    nc = _build_nc()
    in_maps = _stage_inputs(rgb_mask, colors)
    res = run_bass_kernel_spmd(nc, in_maps, core_ids=list(range(NCORES)),
                               trace=trace, **kw)
    return _gather_outputs(res.results), res


def kernel(rgb_mask, colors):
    out, _ = run(rgb_mask, colors)
    return out


# revision 30
# speedup vs baseline: 1.0328x; 1.0328x over previous
"""KNN palette-retrieval kernel for Trainium2 (8 NeuronCores, data-parallel).

Per pixel of rgb_mask [16,3,512,512]: find the palette row (of 21,
L2-normalized) with max cosine similarity, emit that normalized color;
zero pixels emit 0.  argmax(cos) == argmax(dot) since pixel norm is a
positive scalar, so pixel normalization is skipped.

Layout: each core takes 2 batches = 524288 px, split into 32 "sets" g of
16384 px.  PE row layout = 32*k' + g (quadrant-aligned so every DVE
partition range starts at 0/32/64/96).  24 k-slots = 6 matmuls x 4 slots
(21 real + 3 zero-padded; sims are strictly positive so pads never win).

Sims are computed in bf16 hi/lo 3-pass form (x ~ xh+xl, cn ~ ch+cl, all
bf16, split host-side): s = ch.xh + cl.xh + ch.xl accumulated in fp32
PSUM -- ~2^-18 relative accuracy, 3*512 PE cycles per bank instead of
fp32's 4*512, and bf16 weight loads get FWL.

Per tile of 32x512 pixels (six pa banks rotate over seven PSUM pools;
the spare bank lets tile t+1's first mm1 group start while tile t's
max/one-hot tail is still consuming the other banks):
  mm1_i (x6 banks, x3 passes): pa_i = sims         (PE bf16, K=96)
  chain:  s = copy(pa_0) (ACT);  4x running TT-max vs pa_1..pa_4 (DVE)
  fold:   pa_5 as the PSUM operand folds 64 rows across bases, then two
          32-row merges; the last is a scalar_tensor_tensor fusing the
          1e-20 floor and writing into mrep[0:32]; two copies replicate
          it to mrep[32:64] and mrep[64:128].
  bank 0:  mm2 (PE fp32 accumulate, exact: winner row becomes +0.0),
          then oh_0 = Relu(pa_0 * 2.2e7 + 1) on ACT -> bf16 {0,1}.
  banks 1-5: oh_i = (pa_i is_ge mrep) on DVE -> bf16 {0,1} (exact
          compare against the fp32 max; no subtract needed).
  mm3_i (x6): pout[32c+g] += sum cn[k,c]*oh_i  (bf16 PE) -> palette color
  yout:   copy pout -> SBUF (ACT), DMA out.
"""

import sys

sys.path.insert(0, "/opt/trn_rl_repo")

import numpy as np

B, C, H, W = 16, 3, 512, 512
K = 21
NCORES = 8
BPC = B // NCORES            # batches per core
PXC = BPC * H * W            # pixels per core = 524288
G = 32                       # pixel sets (partition-packed)
REG = PXC // G               # 16384 columns per set
NT = 512                     # pixel columns per tile
NTILES = REG // NT           # 32
NMM = 6                      # k-slot matmuls (6*4 = 24 >= 21)
NPOOL = 7                    # PSUM pools for the 6 pa banks: the spare
                             # bank lets tile t+1's mm1 start during
                             # tile t's max/one-hot tail
NSUB = 1                     # number of PE-subtract + ACT-Relu banks
SUB_BANKS = (5,)             # which banks: bank 5, so the latest-freed
                             # PSUM pool feeds tile t+2 (a full tile of
                             # grace) instead of gating tile t+1's bank 1;
                             # banks 0-4 free their pools at DVE is_ge
                             # cadence for the next tile's mm1
OH_SCALE = 2.2e7             # 1 ulp at |m|>=0.25 clears the Relu knee

_CACHE: dict = {}


def _build_nc():
    if "nc" in _CACHE:
        return _CACHE["nc"]
    from contextlib import ExitStack

    import concourse.tile as tile
    from concourse import bacc, mybir

    f32 = mybir.dt.float32
    bf16 = mybir.dt.bfloat16
    mx = mybir.AluOpType.max
    ge = mybir.AluOpType.is_ge
    relu = mybir.ActivationFunctionType.Relu

    nc = bacc.Bacc("TRN2", target_bir_lowering=False, debug=False,
                   num_devices=NCORES)
    xh = nc.dram_tensor("xh", [C * G, REG], bf16, kind="ExternalInput").ap()
    xl = nc.dram_tensor("xl", [C * G, REG], bf16, kind="ExternalInput").ap()
    w1h = nc.dram_tensor("w1h", [NMM, C * G, 128], bf16,
                         kind="ExternalInput").ap()
    w1l = nc.dram_tensor("w1l", [NMM, C * G, 128], bf16,
                         kind="ExternalInput").ap()
    if NSUB:
        w2 = nc.dram_tensor("w2", [G, 128], f32, kind="ExternalInput").ap()
    w3 = nc.dram_tensor("w3", [NMM, 128, C * G], bf16,
                        kind="ExternalInput").ap()
    y = nc.dram_tensor("y", [C * G, REG], f32, kind="ExternalOutput").ap()

    with ExitStack() as ctx:
        tc = ctx.enter_context(tile.TileContext(nc))
        wp = ctx.enter_context(tc.tile_pool(name="w", bufs=1))
        inp = ctx.enter_context(tc.tile_pool(name="xin", bufs=3))
        sp = ctx.enter_context(tc.tile_pool(name="s", bufs=3))
        rp = ctx.enter_context(tc.tile_pool(name="r", bufs=3))
        ohp = ctx.enter_context(tc.tile_pool(name="oh", bufs=3))
        yp = ctx.enter_context(tc.tile_pool(name="y", bufs=3))
        pap = [ctx.enter_context(
            tc.tile_pool(name=f"pa{p}", bufs=1, space="PSUM"))
            for p in range(NPOOL)]
        pop = ctx.enter_context(tc.tile_pool(name="po", bufs=1, space="PSUM"))

        # mm1 weights first (gate tile 0's matmuls); w2/w3 are only needed
        # several microseconds in, so they load behind the first xh/xl.
        w1hs, w1ls, w3s = [], [], []
        for i in range(NMM):
            w1ht = wp.tile([C * G, 128], bf16, name=f"w1hs{i}")
            nc.sync.dma_start(w1ht[:], w1h[i])
            w1hs.append(w1ht)
            w1lt = wp.tile([C * G, 128], bf16, name=f"w1ls{i}")
            nc.sync.dma_start(w1lt[:], w1l[i])
            w1ls.append(w1lt)
        if NSUB:
            w2s = wp.tile([G, 128], f32)
            nc.sync.dma_start(w2s[:], w2[:])
        for i in range(NMM):
            w3t = wp.tile([128, C * G], bf16, name=f"w3s{i}")
            nc.sync.dma_start(w3t[:], w3[i])
            w3s.append(w3t)

        for t in range(NTILES):
            c0 = t * NT
            xht = inp.tile([C * G, NT], bf16, tag="xh")
            nc.sync.dma_start(xht[:], xh[:, c0:c0 + NT])
            xlt = inp.tile([C * G, NT], bf16, tag="xl")
            nc.sync.dma_start(xlt[:], xl[:, c0:c0 + NT])

            pa = []
            for i in range(NMM):
                p = (NMM * t + i) % NPOOL
                pai = pap[p].tile([128, NT], f32, tag=f"pa{p}",
                                  name=f"pa{p}_t{t}")
                nc.tensor.matmul(pai[:], w1hs[i][:], xht[:],
                                 start=True, stop=False)
                nc.tensor.matmul(pai[:], w1ls[i][:], xht[:],
                                 start=False, stop=False)
                nc.tensor.matmul(pai[:], w1hs[i][:], xlt[:],
                                 start=False, stop=False)
                pa.append(pai)

            # running max chain over pa_0..4 (<=1 PSUM operand per TT;
            # SBUF+SBUF inputs must share base partition, PSUM+SBUF is free)
            s = sp.tile([128, NT], f32, tag="s")
            sm = sp.tile([128, NT], f32, tag="sm")
            nc.scalar.copy(s[:], pa[0][:])
            nc.vector.tensor_tensor(sm[:], pa[1][:], s[:], mx)
            nc.vector.tensor_tensor(s[:], pa[2][:], sm[:], mx)
            nc.vector.tensor_tensor(sm[:], pa[3][:], s[:], mx)
            nc.vector.tensor_tensor(s[:], pa[4][:], sm[:], mx)
            # fold: pa_5 (1 real + 3 zero slots) as the PSUM operand lets
            # the 64-row fold cross bases; m12 merges the two 64-row
            # operands at equal base 0 in one TT; a cheap base-rebasing
            # copy then lines up its halves for the floor-fused STT.
            u = sp.tile([64, NT], f32, tag="u")
            nc.vector.tensor_tensor(u[:], pa[5][0:64, :], s[64:128, :], mx)
            m12 = sp.tile([64, NT], f32, tag="m12")
            nc.vector.tensor_tensor(m12[:], u[:], s[0:64, :], mx)
            mc = sp.tile([32, NT], f32, tag="mc")
            nc.vector.tensor_copy(mc[:], m12[32:64, :])
            mrep = rp.tile([128, NT], f32, tag="mrep")
            nc.vector.scalar_tensor_tensor(mrep[0:32, :], mc[:], 1e-20,
                                           m12[0:32, :], mx, mx)
            nc.vector.tensor_copy(mrep[32:64, :], mrep[0:32, :])
            nc.scalar.copy(mrep[64:128, :], mrep[0:64, :])

            for i in SUB_BANKS:
                nc.tensor.matmul(pa[i][:], w2s[:], mrep[0:32, :],
                                 start=False, stop=True)

            # mm3 bank order: is_ge banks first (their oh is ready as soon
            # as mrep lands, before the mm2+Relu banks) and in pool order
            # so the next tile's mm1 banks unlock in sequence.
            pout = pop.tile([C * G, NT], f32, tag="po")
            order = ([i for i in range(NMM) if i not in SUB_BANKS]
                     + list(SUB_BANKS))
            for j, i in enumerate(order):
                oh = ohp.tile([128, NT], bf16, tag=f"oh{i}", name=f"oh{i}")
                if i in SUB_BANKS:
                    nc.scalar.activation(oh[:], pa[i][:], relu,
                                         bias=1.0, scale=OH_SCALE)
                else:
                    nc.vector.tensor_tensor(oh[:], pa[i][:], mrep[:], ge)
                nc.tensor.matmul(pout[:], w3s[i][:], oh[:],
                                 start=(j == 0), stop=(j == NMM - 1))

            yout = yp.tile([C * G, NT], f32, tag="yout")
            nc.scalar.copy(yout[:], pout[:])
            nc.sync.dma_start(y[:, c0:c0 + NT], yout[:])

    nc.compile()
    _CACHE["nc"] = nc
    return nc


def _weights(colors: np.ndarray):
    import ml_dtypes

    bf = ml_dtypes.bfloat16
    cn = (colors.astype(np.float64)
          / np.linalg.norm(colors.astype(np.float64), axis=-1, keepdims=True))
    W1 = np.zeros((NMM, C * G, 128), np.float32)
    W2 = np.zeros((G, 128), np.float32)
    W3 = np.zeros((NMM, 128, C * G), np.float32)
    for i in range(NMM):
        for kp in range(4):
            k = 4 * i + kp
            if k >= K:
                continue
            for g in range(G):
                for c in range(C):
                    W1[i, G * c + g, G * kp + g] = cn[k, c]
                    W3[i, G * kp + g, G * c + g] = cn[k, c]
    for g in range(G):
        for kp in range(4):
            W2[g, G * kp + g] = -1.0
    W1h = W1.astype(bf)
    W1l = (W1 - W1h.astype(np.float32)).astype(bf)
    return W1h, W1l, W2, W3.astype(bf)


def _stage_inputs(rgb_mask: np.ndarray, colors: np.ndarray):
    import ml_dtypes

    bf = ml_dtypes.bfloat16
    W1h, W1l, W2, W3 = _weights(np.asarray(colors, np.float32))
    in_maps = []
    for i in range(NCORES):
        xc = np.asarray(rgb_mask[BPC * i:BPC * (i + 1)], np.float32)
        xc = np.ascontiguousarray(
            np.transpose(xc, (1, 0, 2, 3)).reshape(C * G, REG))
        xh = xc.astype(bf)
        xl = (xc - xh.astype(np.float32)).astype(bf)
        im = {"xh": xh, "xl": xl, "w1h": W1h, "w1l": W1l, "w3": W3}
        if NSUB:
            im["w2"] = W2
        in_maps.append(im)
    return in_maps


def _gather_outputs(results):
    outs = []
    for i in range(NCORES):
        yb = results[i]["y"].reshape(C, BPC, H, W)
        outs.append(np.transpose(yb, (1, 0, 2, 3)))
    return np.ascontiguousarray(np.concatenate(outs, axis=0))


def run(rgb_mask, colors, trace=False, **kw):
    from concourse.bass_utils import run_bass_kernel_spmd

    nc = _build_nc()
    in_maps = _stage_inputs(rgb_mask, colors)
    res = run_bass_kernel_spmd(nc, in_maps, core_ids=list(range(NCORES)),
                               trace=trace, **kw)
    return _gather_outputs(res.results), res


def kernel(rgb_mask, colors):
    out, _ = run(rgb_mask, colors)
    return out


# revision 32
# speedup vs baseline: 1.0358x; 1.0028x over previous
"""KNN palette-retrieval kernel for Trainium2 (8 NeuronCores, data-parallel).

Per pixel of rgb_mask [16,3,512,512]: find the palette row (of 21,
L2-normalized) with max cosine similarity, emit that normalized color;
zero pixels emit 0.  argmax(cos) == argmax(dot) since pixel norm is a
positive scalar, so pixel normalization is skipped.

Layout: each core takes 2 batches = 524288 px, split into 32 "sets" g of
16384 px.  PE row layout = 32*k' + g (quadrant-aligned so every DVE
partition range starts at 0/32/64/96).  24 k-slots = 6 matmuls x 4 slots
(21 real + 3 zero-padded; sims are strictly positive so pads never win).

Sims are computed in bf16 hi/lo 3-pass form (x ~ xh+xl, cn ~ ch+cl, all
bf16, split host-side): s = ch.xh + cl.xh + ch.xl accumulated in fp32
PSUM -- ~2^-18 relative accuracy, 3*512 PE cycles per bank instead of
fp32's 4*512, and bf16 weight loads get FWL.

Per tile of 32x512 pixels (six pa banks rotate over seven PSUM pools;
the spare bank lets tile t+1's first mm1 group start while tile t's
max/one-hot tail is still consuming the other banks):
  mm1_i (x6 banks, x3 passes): pa_i = sims         (PE bf16, K=96)
  chain:  s = copy(pa_0) (ACT);  4x running TT-max vs pa_1..pa_4 (DVE)
  fold:   pa_5 as the PSUM operand folds 64 rows across bases, then two
          32-row merges; the last is a scalar_tensor_tensor fusing the
          1e-20 floor and writing into mrep[0:32]; two copies replicate
          it to mrep[32:64] and mrep[64:128].
  bank 0:  mm2 (PE fp32 accumulate, exact: winner row becomes +0.0),
          then oh_0 = Relu(pa_0 * 2.2e7 + 1) on ACT -> bf16 {0,1}.
  banks 1-5: oh_i = (pa_i is_ge mrep) on DVE -> bf16 {0,1} (exact
          compare against the fp32 max; no subtract needed).
  mm3_i (x6): pout[32c+g] += sum cn[k,c]*oh_i  (bf16 PE) -> palette color
  yout:   copy pout -> SBUF (ACT), DMA out.
"""

import sys

sys.path.insert(0, "/opt/trn_rl_repo")

import numpy as np

B, C, H, W = 16, 3, 512, 512
K = 21
NCORES = 8
BPC = B // NCORES            # batches per core
PXC = BPC * H * W            # pixels per core = 524288
G = 32                       # pixel sets (partition-packed)
REG = PXC // G               # 16384 columns per set
NT = 512                     # pixel columns per tile
NTILES = REG // NT           # 32
NMM = 6                      # k-slot matmuls (6*4 = 24 >= 21)
NPOOL = 7                    # PSUM pools for the 6 pa banks: the spare
                             # bank lets tile t+1's mm1 start during
                             # tile t's max/one-hot tail
NSUB = 1                     # number of PE-subtract + ACT-Relu banks
SUB_BANKS = (0,)             # which banks: bank 0 measured best (bank 5
                             # was a wash; one Relu bank beats 0 or 2)
OH_SCALE = 2.2e7             # 1 ulp at |m|>=0.25 clears the Relu knee

_CACHE: dict = {}


def _build_nc():
    if "nc" in _CACHE:
        return _CACHE["nc"]
    from contextlib import ExitStack

    import concourse.tile as tile
    from concourse import bacc, mybir

    f32 = mybir.dt.float32
    bf16 = mybir.dt.bfloat16
    mx = mybir.AluOpType.max
    ge = mybir.AluOpType.is_ge
    relu = mybir.ActivationFunctionType.Relu

    nc = bacc.Bacc("TRN2", target_bir_lowering=False, debug=False,
                   num_devices=NCORES)
    xh = nc.dram_tensor("xh", [C * G, REG], bf16, kind="ExternalInput").ap()
    xl = nc.dram_tensor("xl", [C * G, REG], bf16, kind="ExternalInput").ap()
    w1h = nc.dram_tensor("w1h", [NMM, C * G, 128], bf16,
                         kind="ExternalInput").ap()
    w1l = nc.dram_tensor("w1l", [NMM, C * G, 128], bf16,
                         kind="ExternalInput").ap()
    if NSUB:
        w2 = nc.dram_tensor("w2", [G, 128], f32, kind="ExternalInput").ap()
    w3 = nc.dram_tensor("w3", [NMM, 128, C * G], bf16,
                        kind="ExternalInput").ap()
    y = nc.dram_tensor("y", [C * G, REG], f32, kind="ExternalOutput").ap()

    with ExitStack() as ctx:
        tc = ctx.enter_context(tile.TileContext(nc))
        wp = ctx.enter_context(tc.tile_pool(name="w", bufs=1))
        inp = ctx.enter_context(tc.tile_pool(name="xin", bufs=6))
        sp = ctx.enter_context(tc.tile_pool(name="s", bufs=4))
        rp = ctx.enter_context(tc.tile_pool(name="r", bufs=4))
        ohp = ctx.enter_context(tc.tile_pool(name="oh", bufs=4))
        yp = ctx.enter_context(tc.tile_pool(name="y", bufs=4))
        pap = [ctx.enter_context(
            tc.tile_pool(name=f"pa{p}", bufs=1, space="PSUM"))
            for p in range(NPOOL)]
        pop = ctx.enter_context(tc.tile_pool(name="po", bufs=1, space="PSUM"))

        # mm1 weights first (gate tile 0's matmuls); w2/w3 are only needed
        # several microseconds in, so they load behind the first xh/xl.
        w1hs, w1ls, w3s = [], [], []
        for i in range(NMM):
            w1ht = wp.tile([C * G, 128], bf16, name=f"w1hs{i}")
            nc.sync.dma_start(w1ht[:], w1h[i])
            w1hs.append(w1ht)
            w1lt = wp.tile([C * G, 128], bf16, name=f"w1ls{i}")
            nc.sync.dma_start(w1lt[:], w1l[i])
            w1ls.append(w1lt)
        if NSUB:
            w2s = wp.tile([G, 128], f32)
            nc.sync.dma_start(w2s[:], w2[:])
        for i in range(NMM):
            w3t = wp.tile([128, C * G], bf16, name=f"w3s{i}")
            nc.sync.dma_start(w3t[:], w3[i])
            w3s.append(w3t)

        for t in range(NTILES):
            c0 = t * NT
            xht = inp.tile([C * G, NT], bf16, tag="xh")
            nc.sync.dma_start(xht[:], xh[:, c0:c0 + NT])
            xlt = inp.tile([C * G, NT], bf16, tag="xl")
            nc.sync.dma_start(xlt[:], xl[:, c0:c0 + NT])

            pa = []
            for i in range(NMM):
                p = (NMM * t + i) % NPOOL
                pai = pap[p].tile([128, NT], f32, tag=f"pa{p}",
                                  name=f"pa{p}_t{t}")
                nc.tensor.matmul(pai[:], w1hs[i][:], xht[:],
                                 start=True, stop=False)
                nc.tensor.matmul(pai[:], w1ls[i][:], xht[:],
                                 start=False, stop=False)
                nc.tensor.matmul(pai[:], w1hs[i][:], xlt[:],
                                 start=False, stop=False)
                pa.append(pai)

            # running max chain over pa_0..4 (<=1 PSUM operand per TT;
            # SBUF+SBUF inputs must share base partition, PSUM+SBUF is free)
            s = sp.tile([128, NT], f32, tag="s")
            sm = sp.tile([128, NT], f32, tag="sm")
            nc.scalar.copy(s[:], pa[0][:])
            nc.vector.tensor_tensor(sm[:], pa[1][:], s[:], mx)
            nc.vector.tensor_tensor(s[:], pa[2][:], sm[:], mx)
            nc.vector.tensor_tensor(sm[:], pa[3][:], s[:], mx)
            nc.vector.tensor_tensor(s[:], pa[4][:], sm[:], mx)
            # fold: pa_5 (1 real + 3 zero slots) as the PSUM operand lets
            # the 64-row fold cross bases; m12 merges the two 64-row
            # operands at equal base 0 in one TT; a cheap base-rebasing
            # copy then lines up its halves for the floor-fused STT.
            u = sp.tile([64, NT], f32, tag="u")
            nc.vector.tensor_tensor(u[:], pa[5][0:64, :], s[64:128, :], mx)
            m12 = sp.tile([64, NT], f32, tag="m12")
            nc.vector.tensor_tensor(m12[:], u[:], s[0:64, :], mx)
            mc = sp.tile([32, NT], f32, tag="mc")
            nc.vector.tensor_copy(mc[:], m12[32:64, :])
            mrep = rp.tile([128, NT], f32, tag="mrep")
            nc.vector.scalar_tensor_tensor(mrep[0:32, :], mc[:], 1e-20,
                                           m12[0:32, :], mx, mx)
            nc.vector.tensor_copy(mrep[32:64, :], mrep[0:32, :])
            nc.scalar.copy(mrep[64:128, :], mrep[0:64, :])

            for i in SUB_BANKS:
                nc.tensor.matmul(pa[i][:], w2s[:], mrep[0:32, :],
                                 start=False, stop=True)

            # mm3 bank order: is_ge banks first (their oh is ready as soon
            # as mrep lands, before the mm2+Relu banks) and in pool order
            # so the next tile's mm1 banks unlock in sequence.
            pout = pop.tile([C * G, NT], f32, tag="po")
            order = ([i for i in range(NMM) if i not in SUB_BANKS]
                     + list(SUB_BANKS))
            for j, i in enumerate(order):
                oh = ohp.tile([128, NT], bf16, tag=f"oh{i}", name=f"oh{i}")
                if i in SUB_BANKS:
                    nc.scalar.activation(oh[:], pa[i][:], relu,
                                         bias=1.0, scale=OH_SCALE)
                else:
                    nc.vector.tensor_tensor(oh[:], pa[i][:], mrep[:], ge)
                nc.tensor.matmul(pout[:], w3s[i][:], oh[:],
                                 start=(j == 0), stop=(j == NMM - 1))

            yout = yp.tile([C * G, NT], f32, tag="yout")
            nc.scalar.copy(yout[:], pout[:])
            nc.sync.dma_start(y[:, c0:c0 + NT], yout[:])

    nc.compile()
    _CACHE["nc"] = nc
    return nc


def _weights(colors: np.ndarray):
    import ml_dtypes

    bf = ml_dtypes.bfloat16
    cn = (colors.astype(np.float64)
          / np.linalg.norm(colors.astype(np.float64), axis=-1, keepdims=True))
    W1 = np.zeros((NMM, C * G, 128), np.float32)
    W2 = np.zeros((G, 128), np.float32)
    W3 = np.zeros((NMM, 128, C * G), np.float32)
    for i in range(NMM):
        for kp in range(4):
            k = 4 * i + kp
            if k >= K:
                continue
            for g in range(G):
                for c in range(C):
                    W1[i, G * c + g, G * kp + g] = cn[k, c]
                    W3[i, G * kp + g, G * c + g] = cn[k, c]
    for g in range(G):
        for kp in range(4):
            W2[g, G * kp + g] = -1.0
    W1h = W1.astype(bf)
    W1l = (W1 - W1h.astype(np.float32)).astype(bf)
    return W1h, W1l, W2, W3.astype(bf)


def _stage_inputs(rgb_mask: np.ndarray, colors: np.ndarray):
    import ml_dtypes

    bf = ml_dtypes.bfloat16
    W1h, W1l, W2, W3 = _weights(np.asarray(colors, np.float32))
    in_maps = []
    for i in range(NCORES):
        xc = np.asarray(rgb_mask[BPC * i:BPC * (i + 1)], np.float32)
        xc = np.ascontiguousarray(
            np.transpose(xc, (1, 0, 2, 3)).reshape(C * G, REG))
        xh = xc.astype(bf)
        xl = (xc - xh.astype(np.float32)).astype(bf)
        im = {"xh": xh, "xl": xl, "w1h": W1h, "w1l": W1l, "w3": W3}
        if NSUB:
            im["w2"] = W2
        in_maps.append(im)
    return in_maps


def _gather_outputs(results):
    outs = []
    for i in range(NCORES):
        yb = results[i]["y"].reshape(C, BPC, H, W)
        outs.append(np.transpose(yb, (1, 0, 2, 3)))
    return np.ascontiguousarray(np.concatenate(outs, axis=0))


def run(rgb_mask, colors, trace=False, **kw):
    from concourse.bass_utils import run_bass_kernel_spmd

    nc = _build_nc()
    in_maps = _stage_inputs(rgb_mask, colors)
    res = run_bass_kernel_spmd(nc, in_maps, core_ids=list(range(NCORES)),
                               trace=trace, **kw)
    return _gather_outputs(res.results), res


def kernel(rgb_mask, colors):
    out, _ = run(rgb_mask, colors)
    return out


# revision 33
# speedup vs baseline: 1.0748x; 1.0377x over previous
"""KNN palette-retrieval kernel for Trainium2 (8 NeuronCores, data-parallel).

Per pixel of rgb_mask [16,3,512,512]: find the palette row (of 21,
L2-normalized) with max cosine similarity, emit that normalized color;
zero pixels emit 0.  argmax(cos) == argmax(dot) since pixel norm is a
positive scalar, so pixel normalization is skipped.

Layout: each core takes 2 batches = 524288 px, split into 32 "sets" g of
16384 px.  PE row layout = 32*k' + g (quadrant-aligned so every DVE
partition range starts at 0/32/64/96).  24 k-slots = 6 matmuls x 4 slots
(21 real + 3 zero-padded; sims are strictly positive so pads never win).

Sims are computed in bf16 hi/lo 3-pass form (x ~ xh+xl, cn ~ ch+cl, all
bf16, split host-side): s = ch.xh + cl.xh + ch.xl accumulated in fp32
PSUM -- ~2^-18 relative accuracy, 3*512 PE cycles per bank instead of
fp32's 4*512, and bf16 weight loads get FWL.

Per tile of 32x512 pixels (six pa banks rotate over seven PSUM pools;
the spare bank lets tile t+1's first mm1 group start while tile t's
max/one-hot tail is still consuming the other banks):
  mm1_i (x6 banks, x3 passes): pa_i = sims         (PE bf16, K=96)
  chain:  s = copy(pa_0) (ACT);  4x running TT-max vs pa_1..pa_4 (DVE)
  fold:   pa_5 as the PSUM operand folds 64 rows across bases, then two
          32-row merges; the last is a scalar_tensor_tensor fusing the
          1e-20 floor and writing into mrep[0:32]; two copies replicate
          it to mrep[32:64] and mrep[64:128].
  bank 0:  mm2 (PE fp32 accumulate, exact: winner row becomes +0.0),
          then oh_0 = Relu(pa_0 * 2.2e7 + 1) on ACT -> bf16 {0,1}.
  banks 1-5: oh_i = (pa_i is_ge mrep) on DVE -> bf16 {0,1} (exact
          compare against the fp32 max; no subtract needed).
  mm3_i (x6): pout[32c+g] += sum cn[k,c]*oh_i  (bf16 PE) -> palette color
  yout:   copy pout -> SBUF (ACT), DMA out.
"""

import sys

sys.path.insert(0, "/opt/trn_rl_repo")

import numpy as np

B, C, H, W = 16, 3, 512, 512
K = 21
NCORES = 8
BPC = B // NCORES            # batches per core
PXC = BPC * H * W            # pixels per core = 524288
G = 32                       # pixel sets (partition-packed)
REG = PXC // G               # 16384 columns per set
NT = 512                     # pixel columns per tile
NTILES = REG // NT           # 32
NMM = 6                      # k-slot matmuls (6*4 = 24 >= 21)
NPOOL = 7                    # PSUM pools for the 6 pa banks: the spare
                             # bank lets tile t+1's mm1 start during
                             # tile t's max/one-hot tail
NSUB = 1                     # number of PE-subtract + ACT-Relu banks
SUB_BANKS = (0,)             # which banks: bank 0 measured best (bank 5
                             # was a wash; one Relu bank beats 0 or 2)
OH_SCALE = 2.2e7             # 1 ulp at |m|>=0.25 clears the Relu knee

_CACHE: dict = {}


def _build_nc():
    if "nc" in _CACHE:
        return _CACHE["nc"]
    from contextlib import ExitStack

    import concourse.tile as tile
    from concourse import bacc, mybir

    f32 = mybir.dt.float32
    bf16 = mybir.dt.bfloat16
    mx = mybir.AluOpType.max
    ge = mybir.AluOpType.is_ge
    relu = mybir.ActivationFunctionType.Relu

    nc = bacc.Bacc("TRN2", target_bir_lowering=False, debug=False,
                   num_devices=NCORES)
    xh = nc.dram_tensor("xh", [C * G, REG], bf16, kind="ExternalInput").ap()
    xl = nc.dram_tensor("xl", [C * G, REG], bf16, kind="ExternalInput").ap()
    w1h = nc.dram_tensor("w1h", [NMM, C * G, 128], bf16,
                         kind="ExternalInput").ap()
    w1l = nc.dram_tensor("w1l", [NMM, C * G, 128], bf16,
                         kind="ExternalInput").ap()
    if NSUB:
        w2 = nc.dram_tensor("w2", [G, 128], f32, kind="ExternalInput").ap()
    w3 = nc.dram_tensor("w3", [NMM, 128, C * G], bf16,
                        kind="ExternalInput").ap()
    y = nc.dram_tensor("y", [C * G, REG], f32, kind="ExternalOutput").ap()

    with ExitStack() as ctx:
        tc = ctx.enter_context(tile.TileContext(nc))
        wp = ctx.enter_context(tc.tile_pool(name="w", bufs=1))
        inp = ctx.enter_context(tc.tile_pool(name="xin", bufs=6))
        sp = ctx.enter_context(tc.tile_pool(name="s", bufs=4))
        rp = ctx.enter_context(tc.tile_pool(name="r", bufs=4))
        ohp = ctx.enter_context(tc.tile_pool(name="oh", bufs=4))
        yp = ctx.enter_context(tc.tile_pool(name="y", bufs=4))
        pap = [ctx.enter_context(
            tc.tile_pool(name=f"pa{p}", bufs=1, space="PSUM"))
            for p in range(NPOOL)]
        pop = ctx.enter_context(tc.tile_pool(name="po", bufs=1, space="PSUM"))

        # mm1 weights first (gate tile 0's matmuls); w2/w3 are only needed
        # several microseconds in, so they load behind the first xh/xl.
        w1hs, w1ls, w3s = [], [], []
        for i in range(NMM):
            w1ht = wp.tile([C * G, 128], bf16, name=f"w1hs{i}")
            nc.sync.dma_start(w1ht[:], w1h[i])
            w1hs.append(w1ht)
            w1lt = wp.tile([C * G, 128], bf16, name=f"w1ls{i}")
            nc.sync.dma_start(w1lt[:], w1l[i])
            w1ls.append(w1lt)
        if NSUB:
            w2s = wp.tile([G, 128], f32)
            nc.sync.dma_start(w2s[:], w2[:])
        for i in range(NMM):
            w3t = wp.tile([128, C * G], bf16, name=f"w3s{i}")
            nc.sync.dma_start(w3t[:], w3[i])
            w3s.append(w3t)

        for t in range(NTILES):
            c0 = t * NT
            xht = inp.tile([C * G, NT], bf16, tag="xh")
            nc.sync.dma_start(xht[:], xh[:, c0:c0 + NT])
            xlt = inp.tile([C * G, NT], bf16, tag="xl")
            nc.sync.dma_start(xlt[:], xl[:, c0:c0 + NT])

            pa = []
            for i in range(NMM):
                p = (NMM * t + i) % NPOOL
                pai = pap[p].tile([128, NT], f32, tag=f"pa{p}",
                                  name=f"pa{p}_t{t}")
                nc.tensor.matmul(pai[:], w1hs[i][:], xht[:],
                                 start=True, stop=False)
                nc.tensor.matmul(pai[:], w1ls[i][:], xht[:],
                                 start=False, stop=False)
                nc.tensor.matmul(pai[:], w1hs[i][:], xlt[:],
                                 start=False, stop=False)
                pa.append(pai)

            # running max chain over pa_0..4 (<=1 PSUM operand per TT;
            # SBUF+SBUF inputs must share base partition, PSUM+SBUF is free)
            s = sp.tile([128, NT], f32, tag="s")
            sm = sp.tile([128, NT], f32, tag="sm")
            nc.scalar.copy(s[:], pa[0][:])
            nc.vector.tensor_tensor(sm[:], pa[1][:], s[:], mx)
            nc.vector.tensor_tensor(s[:], pa[2][:], sm[:], mx)
            nc.vector.tensor_tensor(sm[:], pa[3][:], s[:], mx)
            nc.vector.tensor_tensor(s[:], pa[4][:], sm[:], mx)
            # fold: pa_5 (1 real + 3 zero slots) as the PSUM operand lets
            # the 64-row fold cross bases; m12 merges the two 64-row
            # operands at equal base 0 in one TT; a cheap base-rebasing
            # copy then lines up its halves for the floor-fused STT.
            u = sp.tile([64, NT], f32, tag="u")
            nc.vector.tensor_tensor(u[:], pa[5][0:64, :], s[64:128, :], mx)
            m12 = sp.tile([64, NT], f32, tag="m12")
            nc.vector.tensor_tensor(m12[:], u[:], s[0:64, :], mx)
            mc = sp.tile([32, NT], f32, tag="mc")
            nc.vector.tensor_copy(mc[:], m12[32:64, :])
            mrep = rp.tile([128, NT], f32, tag="mrep")
            nc.vector.scalar_tensor_tensor(mrep[0:32, :], mc[:], 1e-20,
                                           m12[0:32, :], mx, mx)
            nc.vector.tensor_copy(mrep[32:64, :], mrep[0:32, :])
            nc.vector.tensor_copy(mrep[64:128, :], mrep[0:64, :])

            for i in SUB_BANKS:
                nc.tensor.matmul(pa[i][:], w2s[:], mrep[0:32, :],
                                 start=False, stop=True)

            # mm3 bank order: is_ge banks first (their oh is ready as soon
            # as mrep lands, before the mm2+Relu banks) and in pool order
            # so the next tile's mm1 banks unlock in sequence.
            pout = pop.tile([C * G, NT], f32, tag="po")
            order = ([i for i in range(NMM) if i not in SUB_BANKS]
                     + list(SUB_BANKS))
            for j, i in enumerate(order):
                oh = ohp.tile([128, NT], bf16, tag=f"oh{i}", name=f"oh{i}")
                if i in SUB_BANKS:
                    nc.scalar.activation(oh[:], pa[i][:], relu,
                                         bias=1.0, scale=OH_SCALE)
                else:
                    nc.vector.tensor_tensor(oh[:], pa[i][:], mrep[:], ge)
                nc.tensor.matmul(pout[:], w3s[i][:], oh[:],
                                 start=(j == 0), stop=(j == NMM - 1))

            yout = yp.tile([C * G, NT], f32, tag="yout")
            nc.scalar.copy(yout[:], pout[:])
            nc.sync.dma_start(y[:, c0:c0 + NT], yout[:])

    nc.compile()
    _CACHE["nc"] = nc
    return nc


def _weights(colors: np.ndarray):
    import ml_dtypes

    bf = ml_dtypes.bfloat16
    cn = (colors.astype(np.float64)
          / np.linalg.norm(colors.astype(np.float64), axis=-1, keepdims=True))
    W1 = np.zeros((NMM, C * G, 128), np.float32)
    W2 = np.zeros((G, 128), np.float32)
    W3 = np.zeros((NMM, 128, C * G), np.float32)
    for i in range(NMM):
        for kp in range(4):
            k = 4 * i + kp
            if k >= K:
                continue
            for g in range(G):
                for c in range(C):
                    W1[i, G * c + g, G * kp + g] = cn[k, c]
                    W3[i, G * kp + g, G * c + g] = cn[k, c]
    for g in range(G):
        for kp in range(4):
            W2[g, G * kp + g] = -1.0
    W1h = W1.astype(bf)
    W1l = (W1 - W1h.astype(np.float32)).astype(bf)
    return W1h, W1l, W2, W3.astype(bf)


def _stage_inputs(rgb_mask: np.ndarray, colors: np.ndarray):
    import ml_dtypes

    bf = ml_dtypes.bfloat16
    W1h, W1l, W2, W3 = _weights(np.asarray(colors, np.float32))
    in_maps = []
    for i in range(NCORES):
        xc = np.asarray(rgb_mask[BPC * i:BPC * (i + 1)], np.float32)
        xc = np.ascontiguousarray(
            np.transpose(xc, (1, 0, 2, 3)).reshape(C * G, REG))
        xh = xc.astype(bf)
        xl = (xc - xh.astype(np.float32)).astype(bf)
        im = {"xh": xh, "xl": xl, "w1h": W1h, "w1l": W1l, "w3": W3}
        if NSUB:
            im["w2"] = W2
        in_maps.append(im)
    return in_maps


def _gather_outputs(results):
    outs = []
    for i in range(NCORES):
        yb = results[i]["y"].reshape(C, BPC, H, W)
        outs.append(np.transpose(yb, (1, 0, 2, 3)))
    return np.ascontiguousarray(np.concatenate(outs, axis=0))


def run(rgb_mask, colors, trace=False, **kw):
    from concourse.bass_utils import run_bass_kernel_spmd

    nc = _build_nc()
    in_maps = _stage_inputs(rgb_mask, colors)
    res = run_bass_kernel_spmd(nc, in_maps, core_ids=list(range(NCORES)),
                               trace=trace, **kw)
    return _gather_outputs(res.results), res


def kernel(rgb_mask, colors):
    out, _ = run(rgb_mask, colors)
    return out
